# revision 30
# baseline (speedup 1.0000x reference)
"""Trainium2 Bass kernel for nn_FFF (fast-feedforward tree routing).

Strategy (data-parallel over 8 cores, batch-sharded). v2 changes vs v1:
  - xbf (bf16 of x) cast on-chip (gpsimd) instead of streamed: -16MB HBM.
  - y emitted bf16, and the leaf axpy is fused with the PSUM->SBUF copy via
    scalar_tensor_tensor reading PSUM directly: -16MB HBM, less DVE+ACT work.
  - pos9 extracted arithmetically from the level-8 mask (k8 via iota dot,
    g8 via accum of the right-child mask) instead of a 512-wide iota dot.
  - lam9 mult moved to gpsimd; 512-sample tiles amortize mask-op overhead
    (levels 0-5 masks computed for 4 blocks in one 3-D op).
Everything else (dense L via f32r + bf16 corrections for levels 0-6, mask
recurrence, C=m*L, transposed mm B over 512 routing nodes, leaf w2 row via
indirect DMA) matches v1; see kernel_v1.py docstring for the math.
"""

import os
import numpy as np
import ml_dtypes
from contextlib import ExitStack

import concourse.bass as bass  # noqa: F401  (AP helpers)
import concourse.tile as tile
from concourse import bacc, mybir
from concourse.bass_utils import run_bass_kernel_spmd
from concourse.masks import make_identity

F32 = mybir.dt.float32
F32R = mybir.dt.float32r
BF16 = mybir.dt.bfloat16

N_CORES = 8
B_FULL, NIN, NOUT = 65536, 1024, 1024
BC = B_FULL // N_CORES          # 8192 samples per core
DEPTH = 10
NN = 1024                        # node positions (0 = dummy, 1..1023 = nodes)
TB = 512                         # sample tile (4 blocks of 128)
NBLK = 128                       # PE output block (samples)

_CACHE = {}


def _build_nc():
    if "nc" in _CACHE:
        return _CACHE["nc"]
    nc = bacc.Bacc("TRN2", target_bir_lowering=False, debug=False,
                   enable_asserts=False, num_devices=N_CORES)

    xt_d = nc.dram_tensor("xt", [NIN, BC], F32R, kind="ExternalInput").ap()
    xbf_d = nc.dram_tensor("xbf", [NIN, BC], BF16, kind="ExternalInput").ap()
    xlo_d = nc.dram_tensor("xlo", [NIN, BC], BF16, kind="ExternalInput").ap()
    w1t_d = nc.dram_tensor("w1t", [NIN, NN], F32R, kind="ExternalInput").ap()
    w1tbf_d = nc.dram_tensor("w1tbf", [NIN, 128], BF16, kind="ExternalInput").ap()
    w1tlo_d = nc.dram_tensor("w1tlo", [NIN, 128], BF16, kind="ExternalInput").ap()
    w2_d = nc.dram_tensor("w2", [NN, NOUT], BF16, kind="ExternalInput").ap()
    iota_d = nc.dram_tensor("iotak", [128, 256], BF16, kind="ExternalInput").ap()
    y_d = nc.dram_tensor("y", [BC, NOUT], BF16, kind="ExternalOutput").ap()

    with tile.TileContext(nc) as tc:
        with ExitStack() as ctx:
            statics = ctx.enter_context(tc.tile_pool(name="statics", bufs=1))
            xpool = ctx.enter_context(tc.tile_pool(name="xpool", bufs=2))
            lpool = ctx.enter_context(tc.tile_pool(name="lpool", bufs=2))
            mpool = ctx.enter_context(tc.tile_pool(name="mpool", bufs=2))
            cpool = ctx.enter_context(tc.tile_pool(name="cpool", bufs=2))
            ctpool = ctx.enter_context(tc.tile_pool(name="ctpool", bufs=4))
            ypool = ctx.enter_context(tc.tile_pool(name="ypool", bufs=3))
            psumL = ctx.enter_context(tc.tile_pool(name="psumL", bufs=2, space="PSUM"))
            psumT = ctx.enter_context(tc.tile_pool(name="psumT", bufs=2, space="PSUM"))
            psumY = ctx.enter_context(tc.tile_pool(name="psumY", bufs=2, space="PSUM"))

            ident = statics.tile([128, 128], F32)
            make_identity(nc, ident[:])
            identb = statics.tile([128, 128], BF16)
            nc.vector.tensor_copy(identb[:], ident[:])

            w1t_sb = statics.tile([128, 8, NN], F32R)
            w1t_r = w1t_d.rearrange("(ic p) e -> p ic e", p=128)
            # routing half first, per-ic, so mm A can start ASAP
            for ic in range(8):
                nc.scalar.dma_start(w1t_sb[:, ic, 0:512], w1t_r[:, ic, 0:512])
            w1tbf_sb = statics.tile([128, 8, 128], BF16)
            nc.scalar.dma_start(w1tbf_sb[:], w1tbf_d.rearrange("(ic p) e -> p ic e", p=128))
            w1tlo_sb = statics.tile([128, 8, 128], BF16)
            nc.scalar.dma_start(w1tlo_sb[:], w1tlo_d.rearrange("(ic p) e -> p ic e", p=128))
            iota_sb = statics.tile([128, 256], BF16)
            nc.scalar.dma_start(iota_sb[:], iota_d[:])
            nc.scalar.dma_start(w1t_sb[:, :, 512:1024], w1t_r[:, :, 512:1024])
            w2_sb = statics.tile([128, 4, NOUT], BF16)
            nc.scalar.dma_start(w2_sb[:], w2_d[0:512].rearrange("(ec p) n -> p ec n", p=128))

            xt_r = xt_d.rearrange("(ic p) b -> p ic b", p=128)
            xbf_r = xbf_d.rearrange("(ic p) b -> p ic b", p=128)
            xlo_r = xlo_d.rearrange("(ic p) b -> p ic b", p=128)

            n_tiles = BC // TB
            blocks_per_tile = TB // NBLK
            for t in range(n_tiles):
                bsl = slice(t * TB, (t + 1) * TB)
                xt_sb = xpool.tile([128, 8, TB], F32R, tag="xt")
                nc.sync.dma_start(xt_sb[:], xt_r[:, :, bsl])
                xlo_sb = xpool.tile([128, 8, TB], BF16, tag="xlo")
                nc.sync.dma_start(xlo_sb[:], xlo_r[:, :, bsl])
                xbf_sb = xpool.tile([128, 8, TB], BF16, tag="xbf")
                nc.sync.dma_start(xbf_sb[:], xbf_r[:, :, bsl])

                L_sb = lpool.tile([128, blocks_per_tile, NN], F32)
                # ---- mm A ----
                for jb in range(blocks_per_tile):
                    jsl = slice(jb * NBLK, (jb + 1) * NBLK)
                    plr = psumL.tile([128, 512], F32, tag="plr")
                    for ic in range(8):
                        nc.tensor.matmul(
                            plr[:],
                            lhsT=xt_sb[:, ic, jsl],
                            rhs=w1t_sb[:, ic, 0:512],
                            start=(ic == 0), stop=False, skip_group_check=True,
                        )
                    for ic in range(8):
                        nc.tensor.matmul(
                            plr[:, 0:128],
                            lhsT=xlo_sb[:, ic, jsl],
                            rhs=w1tbf_sb[:, ic, :],
                            start=False, stop=False,
                        )
                    for ic in range(8):
                        nc.tensor.matmul(
                            plr[:, 0:128],
                            lhsT=xbf_sb[:, ic, jsl],
                            rhs=w1tlo_sb[:, ic, :],
                            start=False, stop=(ic == 7),
                        )
                    nc.any.tensor_copy(L_sb[:, jb, 0:512], plr[:])
                    # leaf half (positions 512..1023): 1 term
                    pll = psumL.tile([128, 512], F32, tag="pll")
                    for ic in range(8):
                        nc.tensor.matmul(
                            pll[:],
                            lhsT=xt_sb[:, ic, jsl],
                            rhs=w1t_sb[:, ic, 512:1024],
                            start=(ic == 0), stop=(ic == 7),
                        )
                    nc.any.tensor_copy(L_sb[:, jb, 512:1024], pll[:])

                # ---- routing masks ----
                gt_sb = mpool.tile([128, blocks_per_tile, 512], BF16, tag="gt")
                m_sb = mpool.tile([128, blocks_per_tile, NN], BF16, tag="m")
                nc.vector.tensor_single_scalar(
                    gt_sb[:], L_sb[:, :, 0:512], 0.0, mybir.AluOpType.is_gt)
                nc.vector.memset(m_sb[:, :, 0:2], 0.0)
                nc.vector.memset(m_sb[:, :, 1:2], 1.0)
                for d in range(6):   # small levels: one 3-D op covers all blocks
                    sv = 2 ** d
                    n = 2 ** d
                    nc.vector.tensor_mul(
                        m_sb[:, :, 2 * sv + n: 2 * sv + 2 * n],
                        m_sb[:, :, sv: sv + n], gt_sb[:, :, sv: sv + n])
                    nc.vector.tensor_sub(
                        m_sb[:, :, 2 * sv: 2 * sv + n],
                        m_sb[:, :, sv: sv + n],
                        m_sb[:, :, 2 * sv + n: 2 * sv + 2 * n])
                for jb in range(blocks_per_tile):
                    for d in range(6, DEPTH - 1):   # big levels: 1-D per block
                        sv = 2 ** d
                        n = 2 ** d
                        nc.vector.tensor_mul(
                            m_sb[:, jb, 2 * sv + n: 2 * sv + 2 * n],
                            m_sb[:, jb, sv: sv + n], gt_sb[:, jb, sv: sv + n])
                        nc.vector.tensor_sub(
                            m_sb[:, jb, 2 * sv: 2 * sv + n],
                            m_sb[:, jb, sv: sv + n],
                            m_sb[:, jb, 2 * sv + n: 2 * sv + 2 * n])

                # ---- leaf (level 9): pos9 = 512 + k8 + 256*g8, lam9 = <m9, L9> ----
                scrk = mpool.tile([128, 256], BF16, tag="scrk")
                scr9 = mpool.tile([128, 512], F32, tag="scr9")
                trash9 = mpool.tile([128, 512], F32, tag="trash9")
                k8 = mpool.tile([128, blocks_per_tile, 1], F32, tag="k8")
                g8 = mpool.tile([128, blocks_per_tile, 1], F32, tag="g8")
                pos9f = mpool.tile([128, blocks_per_tile, 1], F32, tag="pos9f")
                lam9 = mpool.tile([128, blocks_per_tile, 1], F32, tag="lam9")
                pos9i = mpool.tile([128, blocks_per_tile, 1], mybir.dt.int32, tag="pos9i")
                for jb in range(blocks_per_tile):
                    nc.vector.tensor_mul(scrk[:], m_sb[:, jb, 256:512], iota_sb[:])
                    nc.vector.tensor_reduce(k8[:, jb, :], scrk[:],
                                            axis=mybir.AxisListType.X,
                                            op=mybir.AluOpType.add)
                    nc.vector.tensor_reduce(g8[:, jb, :], m_sb[:, jb, 768:1024],
                                            axis=mybir.AxisListType.X,
                                            op=mybir.AluOpType.add)
                    nc.vector.tensor_mul(scr9[:], m_sb[:, jb, 512:1024],
                                         L_sb[:, jb, 512:1024])
                    nc.scalar.activation(trash9[:], scr9[:],
                                         mybir.ActivationFunctionType.Copy,
                                         accum_out=lam9[:, jb, :])
                    nc.vector.scalar_tensor_tensor(
                        out=pos9f[:, jb, :], in0=g8[:, jb, :], scalar=256.0,
                        in1=k8[:, jb, :],
                        op0=mybir.AluOpType.mult, op1=mybir.AluOpType.add)
                nc.vector.tensor_single_scalar(
                    pos9f[:], pos9f[:], 512.0, mybir.AluOpType.add)
                nc.vector.tensor_copy(pos9i[:], pos9f[:])

                # ---- C = m * L (routing positions only) ----
                C_sb = cpool.tile([128, blocks_per_tile, 512], BF16)
                nc.vector.tensor_mul(C_sb[:], m_sb[:, :, 0:512], L_sb[:, :, 0:512])

                # ---- transpose C, gather leaf w2 row, mm B, fused leaf axpy ----
                for jb in range(blocks_per_tile):
                    ct_sb = ctpool.tile([128, 4, 128], BF16, tag="ct")
                    pt = psumT.tile([128, 512], BF16)
                    for k in range(4):
                        nc.tensor.transpose(
                            pt[:, k * 128:(k + 1) * 128],
                            C_sb[:, jb, k * 128:(k + 1) * 128], identb[:])
                    nc.any.tensor_copy(
                        ct_sb[:].rearrange("p a b -> p (a b)"), pt[:])

                    w2g = ctpool.tile([128, NOUT], BF16, tag="w2g")
                    nc.gpsimd.indirect_dma_start(
                        out=w2g[:], out_offset=None, in_=w2_d[:],
                        in_offset=bass.IndirectOffsetOnAxis(
                            ap=pos9i[:, jb, :], axis=0))

                    y_sb = ypool.tile([128, NOUT], BF16)
                    for nh in range(2):
                        py = psumY.tile([128, 512], F32)
                        for ec in range(4):
                            nc.tensor.matmul(
                                py[:],
                                lhsT=ct_sb[:, ec, :],
                                rhs=w2_sb[:, ec, nh * 512:(nh + 1) * 512],
                                start=(ec == 0), stop=(ec == 3),
                            )
                        nc.vector.scalar_tensor_tensor(
                            out=y_sb[:, nh * 512:(nh + 1) * 512],
                            in0=w2g[:, nh * 512:(nh + 1) * 512],
                            scalar=lam9[:, jb, :], in1=py[:],
                            op0=mybir.AluOpType.mult, op1=mybir.AluOpType.add)
                    nc.sync.dma_start(
                        y_d[t * TB + jb * NBLK: t * TB + (jb + 1) * NBLK, :],
                        y_sb[:])

    nc.compile()
    _CACHE["nc"] = nc
    return nc


def _build_perm():
    """perm[pos-1] = original node id for storage position pos (1..1023)."""
    perm = [0]
    nodes = [0]
    for _ in range(DEPTH - 1):
        nxt = [2 * v + 1 for v in nodes] + [2 * v + 2 for v in nodes]
        perm += nxt
        nodes = nxt
    return np.array(perm, dtype=np.int64)


def _rne11(x):
    """Round-to-nearest-even at 11 mantissa bits (fp32r's operand rounding)."""
    xi = x.view(np.uint32).astype(np.uint64)
    shift = np.uint64(12)
    lsb_mask = np.uint64((1 << 12) - 1)
    half = np.uint64(1 << 11)
    frac = xi & lsb_mask
    base = xi >> shift
    roundup = (frac > half) | ((frac == half) & ((base & np.uint64(1)) == 1))
    out = (base + roundup.astype(np.uint64)) << shift
    return out.astype(np.uint32).view(np.float32)


def kernel(x, w1s, w2s):
    nc = _build_nc()

    perm = _build_perm()
    w1p = np.ascontiguousarray(w1s[perm])          # [1023, 1024]
    w2p = np.ascontiguousarray(w2s[perm])

    w1t = np.zeros((NIN, NN), dtype=np.float32)    # [i, pos]
    w1t[:, 1:] = w1p.T
    w2f = np.zeros((NN, NOUT), dtype=np.float32)
    w2f[1:] = w2p
    w2bf = w2f.astype(ml_dtypes.bfloat16)
    iotak = np.tile(np.arange(256, dtype=np.float32), (128, 1)).astype(
        ml_dtypes.bfloat16)

    w1t_route = w1t[:, 0:128]
    w1tbf = w1t_route.astype(ml_dtypes.bfloat16)
    w1tlo = (w1t_route - _rne11(w1t_route)).astype(ml_dtypes.bfloat16)

    xt = np.ascontiguousarray(x.T)                 # [1024, 65536]
    xbf = xt.astype(ml_dtypes.bfloat16)
    xlo = (xt - _rne11(xt)).astype(ml_dtypes.bfloat16)

    in_maps = []
    for c in range(N_CORES):
        csl = slice(c * BC, (c + 1) * BC)
        in_maps.append({
            "xt": np.ascontiguousarray(xt[:, csl]),
            "xbf": np.ascontiguousarray(xbf[:, csl]),
            "xlo": np.ascontiguousarray(xlo[:, csl]),
            "w1t": w1t, "w1tbf": w1tbf, "w1tlo": w1tlo, "w2": w2bf,
            "iotak": iotak,
        })

    trace = bool(int(os.environ.get("FFF_TRACE", "0")))
    res = run_bass_kernel_spmd(nc, in_maps, core_ids=list(range(N_CORES)),
                               trace=trace)
    _CACHE["last_result"] = res
    y = np.concatenate([res.results[c]["y"].astype(np.float32)
                        for c in range(N_CORES)], axis=0)
    return y


# revision 31
# speedup vs baseline: 1.0094x; 1.0094x over previous
"""Trainium2 Bass kernel for nn_FFF (fast-feedforward tree routing).

Strategy (data-parallel over 8 cores, batch-sharded). v2 changes vs v1:
  - xbf (bf16 of x) cast on-chip (gpsimd) instead of streamed: -16MB HBM.
  - y emitted bf16, and the leaf axpy is fused with the PSUM->SBUF copy via
    scalar_tensor_tensor reading PSUM directly: -16MB HBM, less DVE+ACT work.
  - pos9 extracted arithmetically from the level-8 mask (k8 via iota dot,
    g8 via accum of the right-child mask) instead of a 512-wide iota dot.
  - lam9 mult moved to gpsimd; 512-sample tiles amortize mask-op overhead
    (levels 0-5 masks computed for 4 blocks in one 3-D op).
Everything else (dense L via f32r + bf16 corrections for levels 0-6, mask
recurrence, C=m*L, transposed mm B over 512 routing nodes, leaf w2 row via
indirect DMA) matches v1; see kernel_v1.py docstring for the math.
"""

import os
import numpy as np
import ml_dtypes
from contextlib import ExitStack

import concourse.bass as bass  # noqa: F401  (AP helpers)
import concourse.tile as tile
from concourse import bacc, mybir
from concourse.bass_utils import run_bass_kernel_spmd
from concourse.masks import make_identity

F32 = mybir.dt.float32
F32R = mybir.dt.float32r
BF16 = mybir.dt.bfloat16

N_CORES = 8
B_FULL, NIN, NOUT = 65536, 1024, 1024
BC = B_FULL // N_CORES          # 8192 samples per core
DEPTH = 10
NN = 1024                        # node positions (0 = dummy, 1..1023 = nodes)
TB = 512                         # sample tile (4 blocks of 128)
NBLK = 128                       # PE output block (samples)

_CACHE = {}


def _build_nc():
    if "nc" in _CACHE:
        return _CACHE["nc"]
    nc = bacc.Bacc("TRN2", target_bir_lowering=False, debug=False,
                   enable_asserts=False, num_devices=N_CORES)

    xt_d = nc.dram_tensor("xt", [NIN, BC], F32R, kind="ExternalInput").ap()
    xbf_d = nc.dram_tensor("xbf", [NIN, BC], BF16, kind="ExternalInput").ap()
    xlo_d = nc.dram_tensor("xlo", [NIN, BC], BF16, kind="ExternalInput").ap()
    w1t_d = nc.dram_tensor("w1t", [NIN, NN], F32R, kind="ExternalInput").ap()
    w1tbf_d = nc.dram_tensor("w1tbf", [NIN, 128], BF16, kind="ExternalInput").ap()
    w1tlo_d = nc.dram_tensor("w1tlo", [NIN, 128], BF16, kind="ExternalInput").ap()
    w2_d = nc.dram_tensor("w2", [NN, NOUT], BF16, kind="ExternalInput").ap()
    iota_d = nc.dram_tensor("iotak", [128, 256], BF16, kind="ExternalInput").ap()
    y_d = nc.dram_tensor("y", [BC, NOUT], BF16, kind="ExternalOutput").ap()

    with tile.TileContext(nc) as tc:
        with ExitStack() as ctx:
            statics = ctx.enter_context(tc.tile_pool(name="statics", bufs=1))
            xpool = ctx.enter_context(tc.tile_pool(name="xpool", bufs=2))
            lpool = ctx.enter_context(tc.tile_pool(name="lpool", bufs=2))
            mpool = ctx.enter_context(tc.tile_pool(name="mpool", bufs=2))
            cpool = ctx.enter_context(tc.tile_pool(name="cpool", bufs=2))
            ctpool = ctx.enter_context(tc.tile_pool(name="ctpool", bufs=4))
            ypool = ctx.enter_context(tc.tile_pool(name="ypool", bufs=3))
            psumL = ctx.enter_context(tc.tile_pool(name="psumL", bufs=2, space="PSUM"))
            psumT = ctx.enter_context(tc.tile_pool(name="psumT", bufs=2, space="PSUM"))
            psumY = ctx.enter_context(tc.tile_pool(name="psumY", bufs=2, space="PSUM"))

            ident = statics.tile([128, 128], F32)
            make_identity(nc, ident[:])
            identb = statics.tile([128, 128], BF16)
            nc.vector.tensor_copy(identb[:], ident[:])

            w1t_sb = statics.tile([128, 8, NN], F32R)
            w1t_r = w1t_d.rearrange("(ic p) e -> p ic e", p=128)
            # routing half first so mm A can start before the leaf half lands
            nc.scalar.dma_start(w1t_sb[:, :, 0:512], w1t_r[:, :, 0:512])
            w1tbf_sb = statics.tile([128, 8, 128], BF16)
            nc.scalar.dma_start(w1tbf_sb[:], w1tbf_d.rearrange("(ic p) e -> p ic e", p=128))
            w1tlo_sb = statics.tile([128, 8, 128], BF16)
            nc.scalar.dma_start(w1tlo_sb[:], w1tlo_d.rearrange("(ic p) e -> p ic e", p=128))
            iota_sb = statics.tile([128, 256], BF16)
            nc.scalar.dma_start(iota_sb[:], iota_d[:])
            nc.scalar.dma_start(w1t_sb[:, :, 512:1024], w1t_r[:, :, 512:1024])
            w2_sb = statics.tile([128, 4, NOUT], BF16)
            nc.scalar.dma_start(w2_sb[:], w2_d[0:512].rearrange("(ec p) n -> p ec n", p=128))

            xt_r = xt_d.rearrange("(ic p) b -> p ic b", p=128)
            xbf_r = xbf_d.rearrange("(ic p) b -> p ic b", p=128)
            xlo_r = xlo_d.rearrange("(ic p) b -> p ic b", p=128)

            n_tiles = BC // TB
            blocks_per_tile = TB // NBLK
            for t in range(n_tiles):
                bsl = slice(t * TB, (t + 1) * TB)
                xt_sb = xpool.tile([128, 8, TB], F32R, tag="xt")
                nc.sync.dma_start(xt_sb[:], xt_r[:, :, bsl])
                xlo_sb = xpool.tile([128, 8, TB], BF16, tag="xlo")
                nc.sync.dma_start(xlo_sb[:], xlo_r[:, :, bsl])
                xbf_sb = xpool.tile([128, 8, TB], BF16, tag="xbf")
                nc.sync.dma_start(xbf_sb[:], xbf_r[:, :, bsl])

                L_sb = lpool.tile([128, blocks_per_tile, NN], F32)
                # ---- mm A ----
                for jb in range(blocks_per_tile):
                    jsl = slice(jb * NBLK, (jb + 1) * NBLK)
                    plr = psumL.tile([128, 512], F32, tag="plr")
                    for ic in range(8):
                        nc.tensor.matmul(
                            plr[:],
                            lhsT=xt_sb[:, ic, jsl],
                            rhs=w1t_sb[:, ic, 0:512],
                            start=(ic == 0), stop=False, skip_group_check=True,
                        )
                    for ic in range(8):
                        nc.tensor.matmul(
                            plr[:, 0:128],
                            lhsT=xlo_sb[:, ic, jsl],
                            rhs=w1tbf_sb[:, ic, :],
                            start=False, stop=False,
                        )
                    for ic in range(8):
                        nc.tensor.matmul(
                            plr[:, 0:128],
                            lhsT=xbf_sb[:, ic, jsl],
                            rhs=w1tlo_sb[:, ic, :],
                            start=False, stop=(ic == 7),
                        )
                    nc.any.tensor_copy(L_sb[:, jb, 0:512], plr[:])
                    # leaf half (positions 512..1023): 1 term
                    pll = psumL.tile([128, 512], F32, tag="pll")
                    for ic in range(8):
                        nc.tensor.matmul(
                            pll[:],
                            lhsT=xt_sb[:, ic, jsl],
                            rhs=w1t_sb[:, ic, 512:1024],
                            start=(ic == 0), stop=(ic == 7),
                        )
                    nc.any.tensor_copy(L_sb[:, jb, 512:1024], pll[:])

                # ---- routing masks ----
                gt_sb = mpool.tile([128, blocks_per_tile, 512], BF16, tag="gt")
                m_sb = mpool.tile([128, blocks_per_tile, NN], BF16, tag="m")
                nc.vector.tensor_single_scalar(
                    gt_sb[:], L_sb[:, :, 0:512], 0.0, mybir.AluOpType.is_gt)
                nc.vector.memset(m_sb[:, :, 0:2], 0.0)
                nc.vector.memset(m_sb[:, :, 1:2], 1.0)
                for d in range(6):   # small levels: one 3-D op covers all blocks
                    sv = 2 ** d
                    n = 2 ** d
                    nc.vector.tensor_mul(
                        m_sb[:, :, 2 * sv + n: 2 * sv + 2 * n],
                        m_sb[:, :, sv: sv + n], gt_sb[:, :, sv: sv + n])
                    nc.vector.tensor_sub(
                        m_sb[:, :, 2 * sv: 2 * sv + n],
                        m_sb[:, :, sv: sv + n],
                        m_sb[:, :, 2 * sv + n: 2 * sv + 2 * n])
                for jb in range(blocks_per_tile):
                    for d in range(6, DEPTH - 1):   # big levels: 1-D per block
                        sv = 2 ** d
                        n = 2 ** d
                        nc.vector.tensor_mul(
                            m_sb[:, jb, 2 * sv + n: 2 * sv + 2 * n],
                            m_sb[:, jb, sv: sv + n], gt_sb[:, jb, sv: sv + n])
                        nc.vector.tensor_sub(
                            m_sb[:, jb, 2 * sv: 2 * sv + n],
                            m_sb[:, jb, sv: sv + n],
                            m_sb[:, jb, 2 * sv + n: 2 * sv + 2 * n])

                # ---- leaf (level 9): pos9 = 512 + k8 + 256*g8, lam9 = <m9, L9> ----
                scrk = mpool.tile([128, 256], BF16, tag="scrk")
                trashb = mpool.tile([128, 256], BF16, tag="trashb")
                scr9 = mpool.tile([128, 512], F32, tag="scr9")
                trash9 = mpool.tile([128, 512], F32, tag="trash9")
                k8 = mpool.tile([128, blocks_per_tile, 1], F32, tag="k8")
                g8 = mpool.tile([128, blocks_per_tile, 1], F32, tag="g8")
                pos9f = mpool.tile([128, blocks_per_tile, 1], F32, tag="pos9f")
                lam9 = mpool.tile([128, blocks_per_tile, 1], F32, tag="lam9")
                pos9i = mpool.tile([128, blocks_per_tile, 1], mybir.dt.int32, tag="pos9i")
                for jb in range(blocks_per_tile):
                    nc.vector.tensor_mul(scrk[:], m_sb[:, jb, 256:512], iota_sb[:])
                    nc.scalar.activation(trashb[:], scrk[:],
                                         mybir.ActivationFunctionType.Copy,
                                         accum_out=k8[:, jb, :])
                    nc.scalar.activation(trashb[:], m_sb[:, jb, 768:1024],
                                         mybir.ActivationFunctionType.Copy,
                                         accum_out=g8[:, jb, :])
                    nc.vector.tensor_mul(scr9[:], m_sb[:, jb, 512:1024],
                                         L_sb[:, jb, 512:1024])
                    nc.scalar.activation(trash9[:], scr9[:],
                                         mybir.ActivationFunctionType.Copy,
                                         accum_out=lam9[:, jb, :])
                    nc.vector.scalar_tensor_tensor(
                        out=pos9f[:, jb, :], in0=g8[:, jb, :], scalar=256.0,
                        in1=k8[:, jb, :],
                        op0=mybir.AluOpType.mult, op1=mybir.AluOpType.add)
                nc.vector.tensor_single_scalar(
                    pos9f[:], pos9f[:], 512.0, mybir.AluOpType.add)
                nc.vector.tensor_copy(pos9i[:], pos9f[:])

                # ---- C = m * L (routing positions only) ----
                C_sb = cpool.tile([128, blocks_per_tile, 512], BF16)
                nc.vector.tensor_mul(C_sb[:], m_sb[:, :, 0:512], L_sb[:, :, 0:512])

                # ---- transpose C, gather leaf w2 row, mm B, fused leaf axpy ----
                for jb in range(blocks_per_tile):
                    ct_sb = ctpool.tile([128, 4, 128], BF16, tag="ct")
                    pt = psumT.tile([128, 512], BF16)
                    for k in range(4):
                        nc.tensor.transpose(
                            pt[:, k * 128:(k + 1) * 128],
                            C_sb[:, jb, k * 128:(k + 1) * 128], identb[:])
                    nc.any.tensor_copy(
                        ct_sb[:].rearrange("p a b -> p (a b)"), pt[:])

                    w2g = ctpool.tile([128, NOUT], BF16, tag="w2g")
                    nc.gpsimd.indirect_dma_start(
                        out=w2g[:], out_offset=None, in_=w2_d[:],
                        in_offset=bass.IndirectOffsetOnAxis(
                            ap=pos9i[:, jb, :], axis=0))

                    y_sb = ypool.tile([128, NOUT], BF16)
                    for nh in range(2):
                        py = psumY.tile([128, 512], F32)
                        for ec in range(4):
                            nc.tensor.matmul(
                                py[:],
                                lhsT=ct_sb[:, ec, :],
                                rhs=w2_sb[:, ec, nh * 512:(nh + 1) * 512],
                                start=(ec == 0), stop=(ec == 3),
                            )
                        nc.vector.scalar_tensor_tensor(
                            out=y_sb[:, nh * 512:(nh + 1) * 512],
                            in0=w2g[:, nh * 512:(nh + 1) * 512],
                            scalar=lam9[:, jb, :], in1=py[:],
                            op0=mybir.AluOpType.mult, op1=mybir.AluOpType.add)
                    nc.scalar.dma_start(
                        y_d[t * TB + jb * NBLK: t * TB + (jb + 1) * NBLK, :],
                        y_sb[:])

    nc.compile()
    _CACHE["nc"] = nc
    return nc


def _build_perm():
    """perm[pos-1] = original node id for storage position pos (1..1023)."""
    perm = [0]
    nodes = [0]
    for _ in range(DEPTH - 1):
        nxt = [2 * v + 1 for v in nodes] + [2 * v + 2 for v in nodes]
        perm += nxt
        nodes = nxt
    return np.array(perm, dtype=np.int64)


def _rne11(x):
    """Round-to-nearest-even at 11 mantissa bits (fp32r's operand rounding)."""
    xi = x.view(np.uint32).astype(np.uint64)
    shift = np.uint64(12)
    lsb_mask = np.uint64((1 << 12) - 1)
    half = np.uint64(1 << 11)
    frac = xi & lsb_mask
    base = xi >> shift
    roundup = (frac > half) | ((frac == half) & ((base & np.uint64(1)) == 1))
    out = (base + roundup.astype(np.uint64)) << shift
    return out.astype(np.uint32).view(np.float32)


def kernel(x, w1s, w2s):
    nc = _build_nc()

    perm = _build_perm()
    w1p = np.ascontiguousarray(w1s[perm])          # [1023, 1024]
    w2p = np.ascontiguousarray(w2s[perm])

    w1t = np.zeros((NIN, NN), dtype=np.float32)    # [i, pos]
    w1t[:, 1:] = w1p.T
    w2f = np.zeros((NN, NOUT), dtype=np.float32)
    w2f[1:] = w2p
    w2bf = w2f.astype(ml_dtypes.bfloat16)
    iotak = np.tile(np.arange(256, dtype=np.float32), (128, 1)).astype(
        ml_dtypes.bfloat16)

    w1t_route = w1t[:, 0:128]
    w1tbf = w1t_route.astype(ml_dtypes.bfloat16)
    w1tlo = (w1t_route - _rne11(w1t_route)).astype(ml_dtypes.bfloat16)

    xt = np.ascontiguousarray(x.T)                 # [1024, 65536]
    xbf = xt.astype(ml_dtypes.bfloat16)
    xlo = (xt - _rne11(xt)).astype(ml_dtypes.bfloat16)

    in_maps = []
    for c in range(N_CORES):
        csl = slice(c * BC, (c + 1) * BC)
        in_maps.append({
            "xt": np.ascontiguousarray(xt[:, csl]),
            "xbf": np.ascontiguousarray(xbf[:, csl]),
            "xlo": np.ascontiguousarray(xlo[:, csl]),
            "w1t": w1t, "w1tbf": w1tbf, "w1tlo": w1tlo, "w2": w2bf,
            "iotak": iotak,
        })

    trace = bool(int(os.environ.get("FFF_TRACE", "0")))
    res = run_bass_kernel_spmd(nc, in_maps, core_ids=list(range(N_CORES)),
                               trace=trace)
    _CACHE["last_result"] = res
    y = np.concatenate([res.results[c]["y"].astype(np.float32)
                        for c in range(N_CORES)], axis=0)
    return y


# revision 32
# speedup vs baseline: 1.0282x; 1.0187x over previous
"""Trainium2 Bass kernel for nn_FFF (fast-feedforward tree routing).

Strategy (data-parallel over 8 cores, batch-sharded). v2 changes vs v1:
  - xbf (bf16 of x) cast on-chip (gpsimd) instead of streamed: -16MB HBM.
  - y emitted bf16, and the leaf axpy is fused with the PSUM->SBUF copy via
    scalar_tensor_tensor reading PSUM directly: -16MB HBM, less DVE+ACT work.
  - pos9 extracted arithmetically from the level-8 mask (k8 via iota dot,
    g8 via accum of the right-child mask) instead of a 512-wide iota dot.
  - lam9 mult moved to gpsimd; 512-sample tiles amortize mask-op overhead
    (levels 0-5 masks computed for 4 blocks in one 3-D op).
Everything else (dense L via f32r + bf16 corrections for levels 0-6, mask
recurrence, C=m*L, transposed mm B over 512 routing nodes, leaf w2 row via
indirect DMA) matches v1; see kernel_v1.py docstring for the math.
"""

import os
import numpy as np
import ml_dtypes
from contextlib import ExitStack

import concourse.bass as bass  # noqa: F401  (AP helpers)
import concourse.tile as tile
from concourse import bacc, mybir
from concourse.bass_utils import run_bass_kernel_spmd
from concourse.masks import make_identity

F32 = mybir.dt.float32
F32R = mybir.dt.float32r
BF16 = mybir.dt.bfloat16

N_CORES = 8
B_FULL, NIN, NOUT = 65536, 1024, 1024
BC = B_FULL // N_CORES          # 8192 samples per core
DEPTH = 10
NN = 1024                        # node positions (0 = dummy, 1..1023 = nodes)
TB = 512                         # sample tile (4 blocks of 128)
NBLK = 128                       # PE output block (samples)

_CACHE = {}


def _build_nc():
    if "nc" in _CACHE:
        return _CACHE["nc"]
    nc = bacc.Bacc("TRN2", target_bir_lowering=False, debug=False,
                   enable_asserts=False, num_devices=N_CORES)

    xt_d = nc.dram_tensor("xt", [NIN, BC], F32R, kind="ExternalInput").ap()
    xbf_d = nc.dram_tensor("xbf", [NIN, BC], BF16, kind="ExternalInput").ap()
    xlo_d = nc.dram_tensor("xlo", [NIN, BC], BF16, kind="ExternalInput").ap()
    w1t_d = nc.dram_tensor("w1t", [NIN, NN], F32R, kind="ExternalInput").ap()
    w1tbf_d = nc.dram_tensor("w1tbf", [NIN, 128], BF16, kind="ExternalInput").ap()
    w1tlo_d = nc.dram_tensor("w1tlo", [NIN, 128], BF16, kind="ExternalInput").ap()
    w2_d = nc.dram_tensor("w2", [NN, NOUT], BF16, kind="ExternalInput").ap()
    iota_d = nc.dram_tensor("iotak", [128, 256], BF16, kind="ExternalInput").ap()
    y_d = nc.dram_tensor("y", [BC, NOUT], BF16, kind="ExternalOutput").ap()

    with tile.TileContext(nc) as tc:
        with ExitStack() as ctx:
            statics = ctx.enter_context(tc.tile_pool(name="statics", bufs=1))
            xpool = ctx.enter_context(tc.tile_pool(name="xpool", bufs=2))
            lpool = ctx.enter_context(tc.tile_pool(name="lpool", bufs=2))
            mpool = ctx.enter_context(tc.tile_pool(name="mpool", bufs=2))
            cpool = ctx.enter_context(tc.tile_pool(name="cpool", bufs=2))
            ctpool = ctx.enter_context(tc.tile_pool(name="ctpool", bufs=4))
            ypool = ctx.enter_context(tc.tile_pool(name="ypool", bufs=3))
            psumL = ctx.enter_context(tc.tile_pool(name="psumL", bufs=2, space="PSUM"))
            psumT = ctx.enter_context(tc.tile_pool(name="psumT", bufs=2, space="PSUM"))
            psumY = ctx.enter_context(tc.tile_pool(name="psumY", bufs=2, space="PSUM"))

            ident = statics.tile([128, 128], F32)
            make_identity(nc, ident[:])
            identb = statics.tile([128, 128], BF16)
            nc.vector.tensor_copy(identb[:], ident[:])

            w1t_sb = statics.tile([128, 8, NN], F32R)
            w1t_r = w1t_d.rearrange("(ic p) e -> p ic e", p=128)
            # routing half first, per-ic, so the first mm A starts ASAP
            for ic in range(8):
                nc.scalar.dma_start(w1t_sb[:, ic, 0:512], w1t_r[:, ic, 0:512])
            w1tbf_sb = statics.tile([128, 8, 128], BF16)
            nc.scalar.dma_start(w1tbf_sb[:], w1tbf_d.rearrange("(ic p) e -> p ic e", p=128))
            w1tlo_sb = statics.tile([128, 8, 128], BF16)
            nc.scalar.dma_start(w1tlo_sb[:], w1tlo_d.rearrange("(ic p) e -> p ic e", p=128))
            iota_sb = statics.tile([128, 256], BF16)
            nc.scalar.dma_start(iota_sb[:], iota_d[:])
            nc.scalar.dma_start(w1t_sb[:, :, 512:1024], w1t_r[:, :, 512:1024])
            w2_sb = statics.tile([128, 4, NOUT], BF16)
            nc.scalar.dma_start(w2_sb[:], w2_d[0:512].rearrange("(ec p) n -> p ec n", p=128))

            xt_r = xt_d.rearrange("(ic p) b -> p ic b", p=128)
            xbf_r = xbf_d.rearrange("(ic p) b -> p ic b", p=128)
            xlo_r = xlo_d.rearrange("(ic p) b -> p ic b", p=128)

            n_tiles = BC // TB
            blocks_per_tile = TB // NBLK
            for t in range(n_tiles):
                bsl = slice(t * TB, (t + 1) * TB)
                xt_sb = xpool.tile([128, 8, TB], F32R, tag="xt")
                nc.sync.dma_start(xt_sb[:], xt_r[:, :, bsl])
                xlo_sb = xpool.tile([128, 8, TB], BF16, tag="xlo")
                nc.sync.dma_start(xlo_sb[:], xlo_r[:, :, bsl])
                xbf_sb = xpool.tile([128, 8, TB], BF16, tag="xbf")
                nc.sync.dma_start(xbf_sb[:], xbf_r[:, :, bsl])

                L_sb = lpool.tile([128, blocks_per_tile, NN], F32)
                # ---- mm A ----
                for jb in range(blocks_per_tile):
                    jsl = slice(jb * NBLK, (jb + 1) * NBLK)
                    plr = psumL.tile([128, 512], F32, tag="plr")
                    for ic in range(8):
                        nc.tensor.matmul(
                            plr[:],
                            lhsT=xt_sb[:, ic, jsl],
                            rhs=w1t_sb[:, ic, 0:512],
                            start=(ic == 0), stop=False, skip_group_check=True,
                        )
                    for ic in range(8):
                        nc.tensor.matmul(
                            plr[:, 0:128],
                            lhsT=xlo_sb[:, ic, jsl],
                            rhs=w1tbf_sb[:, ic, :],
                            start=False, stop=False,
                        )
                    for ic in range(8):
                        nc.tensor.matmul(
                            plr[:, 0:128],
                            lhsT=xbf_sb[:, ic, jsl],
                            rhs=w1tlo_sb[:, ic, :],
                            start=False, stop=(ic == 7),
                        )
                    nc.any.tensor_copy(L_sb[:, jb, 0:512], plr[:])
                    # leaf half (positions 512..1023): 1 term
                    pll = psumL.tile([128, 512], F32, tag="pll")
                    for ic in range(8):
                        nc.tensor.matmul(
                            pll[:],
                            lhsT=xt_sb[:, ic, jsl],
                            rhs=w1t_sb[:, ic, 512:1024],
                            start=(ic == 0), stop=(ic == 7),
                        )
                    nc.any.tensor_copy(L_sb[:, jb, 512:1024], pll[:])

                # ---- routing masks ----
                gt_sb = mpool.tile([128, blocks_per_tile, 512], BF16, tag="gt")
                m_sb = mpool.tile([128, blocks_per_tile, NN], BF16, tag="m")
                nc.vector.tensor_single_scalar(
                    gt_sb[:], L_sb[:, :, 0:512], 0.0, mybir.AluOpType.is_gt)
                nc.vector.memset(m_sb[:, :, 0:2], 0.0)
                nc.vector.memset(m_sb[:, :, 1:2], 1.0)
                for d in range(6):   # small levels: one 3-D op covers all blocks
                    sv = 2 ** d
                    n = 2 ** d
                    nc.vector.tensor_mul(
                        m_sb[:, :, 2 * sv + n: 2 * sv + 2 * n],
                        m_sb[:, :, sv: sv + n], gt_sb[:, :, sv: sv + n])
                    nc.vector.tensor_sub(
                        m_sb[:, :, 2 * sv: 2 * sv + n],
                        m_sb[:, :, sv: sv + n],
                        m_sb[:, :, 2 * sv + n: 2 * sv + 2 * n])
                for jb in range(blocks_per_tile):
                    for d in range(6, DEPTH - 1):   # big levels: 1-D per block
                        sv = 2 ** d
                        n = 2 ** d
                        nc.vector.tensor_mul(
                            m_sb[:, jb, 2 * sv + n: 2 * sv + 2 * n],
                            m_sb[:, jb, sv: sv + n], gt_sb[:, jb, sv: sv + n])
                        nc.vector.tensor_sub(
                            m_sb[:, jb, 2 * sv: 2 * sv + n],
                            m_sb[:, jb, sv: sv + n],
                            m_sb[:, jb, 2 * sv + n: 2 * sv + 2 * n])

                # ---- leaf (level 9): pos9 = 512 + k8 + 256*g8, lam9 = <m9, L9> ----
                scrk = mpool.tile([128, 256], BF16, tag="scrk")
                trashb = mpool.tile([128, 256], BF16, tag="trashb")
                scr9 = mpool.tile([128, 512], F32, tag="scr9")
                trash9 = mpool.tile([128, 512], F32, tag="trash9")
                k8 = mpool.tile([128, blocks_per_tile, 1], F32, tag="k8")
                g8 = mpool.tile([128, blocks_per_tile, 1], F32, tag="g8")
                pos9f = mpool.tile([128, blocks_per_tile, 1], F32, tag="pos9f")
                lam9 = mpool.tile([128, blocks_per_tile, 1], F32, tag="lam9")
                pos9i = mpool.tile([128, blocks_per_tile, 1], mybir.dt.int32, tag="pos9i")
                for jb in range(blocks_per_tile):
                    nc.vector.tensor_mul(scrk[:], m_sb[:, jb, 256:512], iota_sb[:])
                    nc.scalar.activation(trashb[:], scrk[:],
                                         mybir.ActivationFunctionType.Copy,
                                         accum_out=k8[:, jb, :])
                    nc.scalar.activation(trashb[:], m_sb[:, jb, 768:1024],
                                         mybir.ActivationFunctionType.Copy,
                                         accum_out=g8[:, jb, :])
                    nc.vector.tensor_mul(scr9[:], m_sb[:, jb, 512:1024],
                                         L_sb[:, jb, 512:1024])
                    nc.scalar.activation(trash9[:], scr9[:],
                                         mybir.ActivationFunctionType.Copy,
                                         accum_out=lam9[:, jb, :])
                    nc.vector.scalar_tensor_tensor(
                        out=pos9f[:, jb, :], in0=g8[:, jb, :], scalar=256.0,
                        in1=k8[:, jb, :],
                        op0=mybir.AluOpType.mult, op1=mybir.AluOpType.add)
                nc.vector.tensor_single_scalar(
                    pos9f[:], pos9f[:], 512.0, mybir.AluOpType.add)
                nc.vector.tensor_copy(pos9i[:], pos9f[:])

                # ---- C = m * L (routing positions only) ----
                C_sb = cpool.tile([128, blocks_per_tile, 512], BF16)
                nc.vector.tensor_mul(C_sb[:], m_sb[:, :, 0:512], L_sb[:, :, 0:512])

                # ---- transpose C, gather leaf w2 row, mm B, fused leaf axpy ----
                for jb in range(blocks_per_tile):
                    ct_sb = ctpool.tile([128, 4, 128], BF16, tag="ct")
                    pt = psumT.tile([128, 512], BF16)
                    for k in range(4):
                        nc.tensor.transpose(
                            pt[:, k * 128:(k + 1) * 128],
                            C_sb[:, jb, k * 128:(k + 1) * 128], identb[:])
                    nc.any.tensor_copy(
                        ct_sb[:].rearrange("p a b -> p (a b)"), pt[:])

                    w2g = ctpool.tile([128, NOUT], BF16, tag="w2g")
                    nc.gpsimd.indirect_dma_start(
                        out=w2g[:], out_offset=None, in_=w2_d[:],
                        in_offset=bass.IndirectOffsetOnAxis(
                            ap=pos9i[:, jb, :], axis=0))

                    y_sb = ypool.tile([128, NOUT], BF16)
                    for nh in range(2):
                        py = psumY.tile([128, 512], F32)
                        for ec in range(4):
                            nc.tensor.matmul(
                                py[:],
                                lhsT=ct_sb[:, ec, :],
                                rhs=w2_sb[:, ec, nh * 512:(nh + 1) * 512],
                                start=(ec == 0), stop=(ec == 3),
                            )
                        nc.vector.scalar_tensor_tensor(
                            out=y_sb[:, nh * 512:(nh + 1) * 512],
                            in0=w2g[:, nh * 512:(nh + 1) * 512],
                            scalar=lam9[:, jb, :], in1=py[:],
                            op0=mybir.AluOpType.mult, op1=mybir.AluOpType.add)
                    nc.scalar.dma_start(
                        y_d[t * TB + jb * NBLK: t * TB + (jb + 1) * NBLK, :],
                        y_sb[:])

    nc.compile()
    _CACHE["nc"] = nc
    return nc


def _build_perm():
    """perm[pos-1] = original node id for storage position pos (1..1023)."""
    perm = [0]
    nodes = [0]
    for _ in range(DEPTH - 1):
        nxt = [2 * v + 1 for v in nodes] + [2 * v + 2 for v in nodes]
        perm += nxt
        nodes = nxt
    return np.array(perm, dtype=np.int64)


def _rne11(x):
    """Round-to-nearest-even at 11 mantissa bits (fp32r's operand rounding)."""
    xi = x.view(np.uint32).astype(np.uint64)
    shift = np.uint64(12)
    lsb_mask = np.uint64((1 << 12) - 1)
    half = np.uint64(1 << 11)
    frac = xi & lsb_mask
    base = xi >> shift
    roundup = (frac > half) | ((frac == half) & ((base & np.uint64(1)) == 1))
    out = (base + roundup.astype(np.uint64)) << shift
    return out.astype(np.uint32).view(np.float32)


def kernel(x, w1s, w2s):
    nc = _build_nc()

    perm = _build_perm()
    w1p = np.ascontiguousarray(w1s[perm])          # [1023, 1024]
    w2p = np.ascontiguousarray(w2s[perm])

    w1t = np.zeros((NIN, NN), dtype=np.float32)    # [i, pos]
    w1t[:, 1:] = w1p.T
    w2f = np.zeros((NN, NOUT), dtype=np.float32)
    w2f[1:] = w2p
    w2bf = w2f.astype(ml_dtypes.bfloat16)
    iotak = np.tile(np.arange(256, dtype=np.float32), (128, 1)).astype(
        ml_dtypes.bfloat16)

    w1t_route = w1t[:, 0:128]
    w1tbf = w1t_route.astype(ml_dtypes.bfloat16)
    w1tlo = (w1t_route - _rne11(w1t_route)).astype(ml_dtypes.bfloat16)

    xt = np.ascontiguousarray(x.T)                 # [1024, 65536]
    xbf = xt.astype(ml_dtypes.bfloat16)
    xlo = (xt - _rne11(xt)).astype(ml_dtypes.bfloat16)

    in_maps = []
    for c in range(N_CORES):
        csl = slice(c * BC, (c + 1) * BC)
        in_maps.append({
            "xt": np.ascontiguousarray(xt[:, csl]),
            "xbf": np.ascontiguousarray(xbf[:, csl]),
            "xlo": np.ascontiguousarray(xlo[:, csl]),
            "w1t": w1t, "w1tbf": w1tbf, "w1tlo": w1tlo, "w2": w2bf,
            "iotak": iotak,
        })

    trace = bool(int(os.environ.get("FFF_TRACE", "0")))
    res = run_bass_kernel_spmd(nc, in_maps, core_ids=list(range(N_CORES)),
                               trace=trace)
    _CACHE["last_result"] = res
    y = np.concatenate([res.results[c]["y"].astype(np.float32)
                        for c in range(N_CORES)], axis=0)
    return y
